# revision 31
# baseline (speedup 1.0000x reference)
"""Trainium2 Bass kernel for nn_MultiHeadAttention (B=2, S=2048, D=1024, H=16).

Sharding: 8 cores = 2 batches x 4 head-groups. Core c handles batch c//4 and
heads [4*(c%4), 4*(c%4)+4). Each core computes its 4 heads' attention plus the
row-slice of the output projection; the host sums the 4 partial outputs per
batch and adds the output bias.

Per-core structure (all matmuls bf16, fp32 PSUM):
  - qT/kT in [head_dim, seq] layout (host provides x^T), packed 2 heads per
    128-partition tile. wq is host-prescaled by 16*log2(e) so the score PSUM
    is 128*log2(e)*scores/8 directly.
  - scoresT[kv, q] = kT.T @ qT per (head, kv-tile-pair, q-chunk); exp2 on
    ScalarE reading PSUM (scale=ln2/128, bias=-4*ln2 folded into the
    activation; the 2^-4 shift cancels in normalization and keeps bf16
    exponents comfortable).
  - attn[q, d] in q-major layout: attnT'[q, d+1] = ex_slice.T @ [v | 1]
    accumulated over the 16 kv tiles in PSUM (kv innermost so only one
    2KB accumulation region is live per (head, q-tile)); col 64 collects
    the softmax denominators.
  - 1/sums via exact DVE reciprocal; normalize+drain to bf16 on GPSIMD with
    a per-partition scalar multiply.
  - attn -> attnT via DMA XBAR transpose (head pairs stacked to 128 rows);
    out[q, :] = attnT.T @ woPair accumulated over the 2 head pairs (K=128).
"""

import sys

for _p in ("/opt/trn_rl_repo",):
    if _p not in sys.path:
        sys.path.insert(0, _p)

import numpy as np
import ml_dtypes

BF16 = ml_dtypes.bfloat16

S = 2048          # sequence length
D = 1024          # embed dim
HC = 4            # heads per core
HD = 64           # head dim
DC = HC * HD      # per-core projection width (256)
ST = S // 128     # kv tiles (16)
DT = D // 128     # D tiles (8)
QC = S // 512     # q chunks of 512 (4)
NCORES = 8

LOG2E = 1.4426950408889634
PRESCALE = 16.0 * LOG2E   # folded into wq on the host
CSHIFT = 4.0              # ex = exp(s/8) * 2^-CSHIFT

_PROGRAM = None


def _build_program():
    import concourse.mybir as mybir
    import concourse.tile as tile
    from concourse import bacc

    dt = mybir.dt
    AF = mybir.ActivationFunctionType
    ALU = mybir.AluOpType

    nc = bacc.Bacc()

    xqT = nc.declare_dram_parameter("xqT", [D, S], dt.bfloat16, isOutput=False)
    xkT = nc.declare_dram_parameter("xkT", [D, S], dt.bfloat16, isOutput=False)
    xvT = nc.declare_dram_parameter("xvT", [D, S], dt.bfloat16, isOutput=False)
    wq = nc.declare_dram_parameter("wq", [D, DC], dt.bfloat16, isOutput=False)
    wk = nc.declare_dram_parameter("wk", [D, DC], dt.bfloat16, isOutput=False)
    wv = nc.declare_dram_parameter("wv", [D, DC], dt.bfloat16, isOutput=False)
    # wo pairs: [pair, 128 = (2 heads x 64 hd), D]
    woP = nc.declare_dram_parameter("woP", [128, 2, D], dt.bfloat16, isOutput=False)
    bq = nc.declare_dram_parameter("bq", [128, 2], dt.float32, isOutput=False)
    bk = nc.declare_dram_parameter("bk", [128, 2], dt.float32, isOutput=False)
    bv = nc.declare_dram_parameter("bv", [128, DC], dt.float32, isOutput=False)
    out = nc.declare_dram_parameter("out", [S, D], dt.bfloat16, isOutput=True)

    out_t = out.rearrange("(t p) d -> t p d", p=128)

    EXP_SCALE = float(np.log(2.0) / 128.0)
    EXP_BIAS = float(-CSHIFT * np.log(2.0))

    with tile.TileContext(nc) as tc:
        with (
            tc.tile_pool(name="const", bufs=1) as cp,
            tc.tile_pool(name="xt", bufs=18) as xp,
            tc.tile_pool(name="xqt", bufs=16) as xqp,
            tc.tile_pool(name="expp", bufs=40) as ep,
            tc.tile_pool(name="atp", bufs=12) as atp,
            tc.tile_pool(name="attp", bufs=16) as atpT,
            tc.tile_pool(name="rcp", bufs=8) as rcp,
            tc.tile_pool(name="outp", bufs=4) as op_,
            tc.tile_pool(name="pa", bufs=2, space="PSUM") as pa,
            tc.tile_pool(name="pv", bufs=2, space="PSUM") as pv,
            tc.tile_pool(name="pb", bufs=2, space="PSUM") as pb,
        ):
            # ---- constants ----
            wq_sb = cp.tile([128, DT, DC], dt.bfloat16, tag="wq_sb")
            wk_sb = cp.tile([128, DT, DC], dt.bfloat16, tag="wk_sb")
            wv_sb = cp.tile([128, DT, DC], dt.bfloat16, tag="wv_sb")
            wo_sb = cp.tile([128, 2, D], dt.bfloat16, tag="wo_sb")
            bq_sb = cp.tile([128, 2], dt.float32, tag="bq_sb")
            bk_sb = cp.tile([128, 2], dt.float32, tag="bk_sb")
            bv_sb = cp.tile([128, HC, HD], dt.float32, tag="bv_sb")
            ebias_sb = cp.tile([128, 1], dt.float32, tag="ebias_sb")
            nc.gpsimd.memset(ebias_sb[:], EXP_BIAS)
            warm_sb = cp.tile([1, 1], dt.float32, tag="warm_sb")
            nc.gpsimd.memset(warm_sb[:], 0.0)
            # tiny warm-up exp so the activation-table load happens at t~0
            # instead of stalling the first real exp tile
            nc.scalar.activation(warm_sb[:], warm_sb[:], AF.Exp)
            nc.sync.dma_start(wk_sb[:], wk.rearrange("(t p) m -> p t m", p=128))
            nc.sync.dma_start(bk_sb[:], bk[:])

            qT_sb = [cp.tile([128, 2, 512], dt.bfloat16, tag=f"qT_sb{i}", name=f"qT_sb{i}") for i in range(QC)]
            kT_sb = [cp.tile([128, 2, 512], dt.bfloat16, tag=f"kT_sb{i}", name=f"kT_sb{i}") for i in range(QC)]
            # v' per kv tile: [128 kv, head, 64 v + ones col]
            v_sb = [cp.tile([128, HC, 65], dt.bfloat16, tag=f"v_sb{i}", name=f"v_sb{i}") for i in range(ST)]
            for st in range(ST):
                nc.gpsimd.memset(v_sb[st][:, :, 64:65], 1.0)

            # ---- projection helpers (x staged as [128, 512] quarter tiles) ----
            def load_xquarter(xT, xts, q, pool=None):
                xr = xT.rearrange("(t p) s -> p t s", p=128)
                for Dti in range(DT):
                    xtile = (pool or xp).tile([128, 512], dt.bfloat16, tag="xt",
                                              name=f"xt_{Dti}_{q}")
                    nc.sync.dma_start(xtile[:], xr[:, Dti, q * 512:(q + 1) * 512])
                    xts[Dti][q] = xtile

            def load_xhalf2(xT, xts, qlo, pool=None):
                xr = xT.rearrange("(t p) s -> p t s", p=128)
                for Dti in range(DT):
                    xtile = (pool or xp).tile([128, 1024], dt.bfloat16, tag="xt",
                                              name=f"xh_{Dti}_{qlo}")
                    nc.sync.dma_start(xtile[:], xr[:, Dti, qlo * 512:(qlo + 2) * 512])
                    xts[Dti][qlo] = xtile[:, 0:512]
                    xts[Dti][qlo + 1] = xtile[:, 512:1024]

            def qk_proj(xts, w_sb, dst, b_sb, qc):
                for pt in range(2):
                    ps = pb.tile([128, 512], dt.float32, tag="pb", name=f"pp_{qc}_{pt}")
                    for Dti in range(DT):
                        nc.tensor.matmul(
                            ps[:],
                            w_sb[:, Dti, pt * 128:(pt + 1) * 128],
                            xts[Dti][qc][:],
                            start=(Dti == 0),
                            stop=(Dti == DT - 1),
                        )
                    nc.vector.tensor_scalar_add(
                        dst[qc][:, pt, :], ps[:], b_sb[:, pt:pt + 1],
                    )

            def v_proj(xts, st_range):
                for st in st_range:
                    q, off = st // 4, (st % 4) * 128
                    ps = pv.tile([128, DC], dt.float32, tag="pv", name=f"vp_{st}")
                    for Dti in range(DT):
                        nc.tensor.matmul(
                            ps[:],
                            xts[Dti][q][:, off:off + 128],
                            wv_sb[:, Dti, :],
                            start=(Dti == 0),
                            stop=(Dti == DT - 1),
                        )
                    nc.vector.tensor_tensor(
                        v_sb[st][:, :, 0:64],
                        ps.rearrange("p (h d) -> p h d", d=HD),
                        bv_sb[:],
                        ALU.add,
                    )

            # ---- attention ----
            ex_tiles = {}

            def scores_exp(qc, h, m):
                """Scores+exp for q-chunk qc, head h, kv-tile pair m (kv tiles 2m, 2m+1)."""
                pt, lo = h // 2, (h % 2) * 64
                scp = pa.tile([128, 2, 512], dt.float32, tag="pa", name=f"sc_{qc}_{h}_{m}")
                for j in range(2):
                    kt = m * 2 + j
                    nc.tensor.matmul(
                        scp[:, j, :],
                        kT_sb[kt // 4][lo:lo + 64, pt, (kt % 4) * 128:(kt % 4 + 1) * 128],
                        qT_sb[qc][lo:lo + 64, pt, :],
                        start=True,
                        stop=True,
                    )
                ex = ep.tile([128, 2, 512], dt.bfloat16, tag="ex", name=f"ex_{qc}_{h}_{m}")
                nc.scalar.activation(ex[:], scp[:], AF.Exp, scale=EXP_SCALE,
                                     bias=ebias_sb[:])
                ex_tiles[(qc, h, m)] = ex

            def attn_head(qc, h, at_pair, atT):
                """attnV + normalize for (qc, h); writes at_pair[qs][:, (h%2)*64:...].
                For the odd head of a pair, fires the pair's transpose per qs
                as soon as its half is written."""
                exs = [ex_tiles.pop((qc, h, m)) for m in range(8)]
                rc = rcp.tile([128, 4], dt.float32, tag="rc", name=f"rc_{qc}_{h}")
                pair = h // 2
                for qs in range(4):
                    pA = pv.tile([128, 512], dt.float32, tag="pv", name=f"att_{qc}_{h}_{qs}")
                    for kt in range(ST):
                        nc.tensor.matmul(
                            pA[:, 0:65],
                            exs[kt // 2][:, kt % 2, qs * 128:(qs + 1) * 128],
                            v_sb[kt][:, h, :],
                            start=(kt == 0),
                            stop=(kt == ST - 1),
                        )
                    nc.vector.reciprocal(rc[:, qs:qs + 1], pA[:, 64:65])
                    nc.vector.tensor_scalar_mul(
                        at_pair[qs][:, (h % 2) * 64:(h % 2) * 64 + 64],
                        pA[:, 0:64],
                        rc[:, qs:qs + 1],
                    )
                    if h % 2 == 1:
                        t = atpT.tile([128, 128], dt.bfloat16, tag="atT",
                                      name=f"atT_{qc}_{pair}_{qs}")
                        nc.sync.dma_start(t[:], at_pair[qs][:], transpose=True)
                        atT[(pair, qs)] = t
                        if qc == QC - 1 and h == 3 and qs >= 1:
                            out_proj(qc, qs - 1, atT)
                if qc == QC - 1 and h == 3:
                    out_proj(qc, 3, atT)

            def out_half(qc, qs, dc2, atT):
                st = qc * 4 + qs
                o_sb = op_.tile([128, 512], dt.bfloat16, tag="osb",
                                name=f"osb_{st}_{dc2}")
                po = pb.tile([128, 512], dt.float32, tag="pb", name=f"po_{st}_{dc2}")
                for pair in range(2):
                    nc.tensor.matmul(
                        po[:],
                        atT[(pair, qs)][:],
                        wo_sb[:, pair, dc2 * 512:(dc2 + 1) * 512],
                        start=(pair == 0),
                        stop=(pair == 1),
                    )
                nc.vector.tensor_copy(o_sb[:], po[:])
                nc.sync.dma_start(
                    out_t[st][:, dc2 * 512:(dc2 + 1) * 512], o_sb[:])

            def out_proj(qc, qs, atT):
                out_half(qc, qs, 0, atT)
                out_half(qc, qs, 1, atT)

            # ---- trace order ----
            # The one modeled HWDGE device serializes all DMAs in issue order:
            # weights, xk-q0/xq-q0 (first scores ~12us in), remaining kT
            # quarters, xq-q1 (feeds qproj1 -> the early qc1 scores batch that
            # bridges the vproj window), then xv, then xq q2/q3.
            xk_ts = [[None] * 4 for _ in range(DT)]
            xq_ts = [[None] * 4 for _ in range(DT)]
            xv_ts = [[None] * 4 for _ in range(DT)]
            nc.sync.dma_start(wq_sb[:], wq.rearrange("(t p) m -> p t m", p=128))
            nc.sync.dma_start(bq_sb[:], bq[:])
            xkr2 = xkT.rearrange("(t p) s -> p t s", p=128)
            xqr2 = xqT.rearrange("(t p) s -> p t s", p=128)
            for Dti in range(DT):
                xkt = xp.tile([128, 1024], dt.bfloat16, tag="xt", name=f"xkh_{Dti}_0")
                nc.sync.dma_start(xkt[:], xkr2[:, Dti, 0:1024])
                xk_ts[Dti][0] = xkt[:, 0:512]
                xk_ts[Dti][1] = xkt[:, 512:1024]
                xqt = xqp.tile([128, 512], dt.bfloat16, tag="xt", name=f"xqq_{Dti}_0")
                nc.sync.dma_start(xqt[:], xqr2[:, Dti, 0:512])
                xq_ts[Dti][0] = xqt
            load_xhalf2(xkT, xk_ts, 2)
            load_xquarter(xqT, xq_ts, 1, pool=xqp)
            nc.sync.dma_start(wv_sb[:], wv.rearrange("(t p) m -> p t m", p=128))
            nc.sync.dma_start(bv_sb[:], bv.rearrange("p (h d) -> p h d", d=HD))
            load_xhalf2(xvT, xv_ts, 0)
            load_xhalf2(xvT, xv_ts, 2)
            nc.sync.dma_start(wo_sb[:], woP[:])
            load_xhalf2(xqT, xq_ts, 2, pool=xqp)
            qk_proj(xk_ts, wk_sb, kT_sb, bk_sb, 0)
            qk_proj(xk_ts, wk_sb, kT_sb, bk_sb, 1)
            qk_proj(xq_ts, wq_sb, qT_sb, bq_sb, 0)
            # qc0 exp backlog: head 0 complete first (the ex ring frees in
            # exactly the order ah(0,*) consumes), m-paced behind kT arrivals
            for m in range(4):
                scores_exp(0, 0, m)
            qk_proj(xk_ts, wk_sb, kT_sb, bk_sb, 2)
            scores_exp(0, 0, 4)
            scores_exp(0, 0, 5)
            qk_proj(xk_ts, wk_sb, kT_sb, bk_sb, 3)
            scores_exp(0, 0, 6)
            scores_exp(0, 0, 7)
            for h in range(1, 4):
                for m in range(8):
                    scores_exp(0, h, m)

            # ---- software pipeline over (qc, h): scores for qc+1 stream
            # between attnV heads of qc so ACT never starves.
            def sc(qc, h, mlo=0, mhi=8):
                for m in range(mlo, mhi):
                    scores_exp(qc, h, m)

            atTs = {qc: {} for qc in range(QC)}
            ats = {}

            def ah(qc, h):
                pair = h // 2
                if (qc, pair) not in ats:
                    ats[(qc, pair)] = [
                        atp.tile([128, 128], dt.bfloat16, tag="at",
                                 name=f"at_{qc}_{pair}_{qs}")
                        for qs in range(4)
                    ]
                attn_head(qc, h, ats[(qc, pair)], atTs[qc])

            # Hand-rolled schedule. The tail interleaves the last two
            # scores batches at scp granularity ahead of the remaining attnV
            # heads so almost no PE work is queued behind the final exps.
            steps = [
                ("qp", 1), ("sc", 1, 0), ("vp",),
                ("ah", 0, 0), ("sc", 1, 1), ("qp", 2),
                ("ah", 0, 1), ("sc", 1, 2), ("qp", 3),
                ("ah", 0, 2), ("sc", 1, 3),
                ("ah", 0, 3), ("sc", 2, 0),
                ("ah", 1, 0), ("oh", 0, 0), ("sc", 2, 1), ("oh", 0, 1),
                ("ah", 1, 1), ("oh", 0, 2), ("sc", 2, 2), ("oh", 0, 3),
                ("ah", 1, 2), ("oh", 0, 4), ("sc", 2, 3), ("oh", 0, 5),
                ("ah", 1, 3), ("oh", 0, 6), ("sc", 3, 0), ("oh", 0, 7),
                ("ah", 2, 0), ("oh", 1, 0), ("sc", 3, 1), ("oh", 1, 1),
                ("ah", 2, 1), ("oh", 1, 2), ("sc", 3, 2, 0, 4), ("oh", 1, 3),
                ("ah", 2, 2), ("oh", 1, 4), ("sc", 3, 2, 4, 8),
                ("sc", 3, 3, 0, 2), ("oh", 1, 5),
                ("ah", 2, 3), ("oh", 1, 6), ("sc", 3, 3, 2, 5), ("oh", 1, 7),
                ("ah", 3, 0), ("oh", 2, 0), ("sc", 3, 3, 5, 8), ("oh", 2, 1),
                ("ah", 3, 1), ("oh", 2, 2), ("oh", 2, 3),
                ("ah", 3, 2), ("oh", 2, 4), ("oh", 2, 5),
                ("oh", 2, 6), ("oh", 2, 7),
                ("ah", 3, 3),
            ]
            for step in steps:
                op = step[0]
                if op == "sc":
                    sc(*step[1:])
                elif op == "ah":
                    ah(step[1], step[2])
                elif op == "oh":
                    out_half(step[1], step[2] // 2, step[2] % 2, atTs[step[1]])
                elif op == "qp":
                    qk_proj(xq_ts, wq_sb, qT_sb, bq_sb, step[1])
                elif op == "vp":
                    v_proj(xv_ts, range(0, ST))

    nc.finalize()
    return nc


def _get_program():
    global _PROGRAM
    if _PROGRAM is None:
        _PROGRAM = _build_program()
    return _PROGRAM


def _prep_core_inputs(x_q, x_k, x_v, wq, bq, wk, bk, wv, bv, wo):
    """Build the 8 per-core input dicts (host-side shard + cast)."""
    xT = {}
    for b in range(2):
        xT[b] = (
            np.ascontiguousarray(x_q[b].T).astype(BF16),
            np.ascontiguousarray(x_k[b].T).astype(BF16),
            np.ascontiguousarray(x_v[b].T).astype(BF16),
        )
    wq_s = (wq * PRESCALE).astype(BF16)
    bq_s = (bq * PRESCALE).astype(np.float32)
    in_maps = []
    for c in range(NCORES):
        b, g = c // 4, c % 4
        sl = slice(g * DC, (g + 1) * DC)
        # wo rows for this head group, head-pairs stacked on partitions:
        # woP[p, pair, :] with p = (h_in_pair * 64 + hd)
        wo_c = np.ascontiguousarray(
            wo[sl, :].reshape(2, 2 * HD, D).transpose(1, 0, 2)
        ).astype(BF16)
        in_maps.append({
            "xqT": xT[b][0],
            "xkT": xT[b][1],
            "xvT": xT[b][2],
            "wq": wq_s[:, sl],
            "wk": wk[:, sl].astype(BF16),
            "wv": wv[:, sl].astype(BF16),
            "woP": wo_c,
            "bq": np.ascontiguousarray(bq_s[sl].reshape(2, 128).T).astype(np.float32),
            "bk": np.ascontiguousarray(bk[sl].reshape(2, 128).T).astype(np.float32),
            "bv": np.broadcast_to(bv[sl], (128, DC)).astype(np.float32).copy(),
        })
    return in_maps


def kernel(x_q, x_k, x_v, wq, bq, wk, bk, wv, bv, wo, bo):
    from concourse.bass_utils import run_bass_kernel_spmd

    x_q = np.asarray(x_q, np.float32)
    x_k = np.asarray(x_k, np.float32)
    x_v = np.asarray(x_v, np.float32)
    wq = np.asarray(wq, np.float32)
    wk = np.asarray(wk, np.float32)
    wv = np.asarray(wv, np.float32)
    wo = np.asarray(wo, np.float32)
    bq = np.asarray(bq, np.float32)
    bk = np.asarray(bk, np.float32)
    bv = np.asarray(bv, np.float32)
    bo = np.asarray(bo, np.float32)

    nc = _get_program()
    in_maps = _prep_core_inputs(x_q, x_k, x_v, wq, bq, wk, bk, wv, bv, wo)
    res = run_bass_kernel_spmd(nc, in_maps, list(range(NCORES)))

    out = np.zeros((2, S, D), np.float32)
    for c in range(NCORES):
        out[c // 4] += res.results[c]["out"].astype(np.float32)
    out += bo
    return out


# revision 32
# speedup vs baseline: 1.0207x; 1.0207x over previous
"""Trainium2 Bass kernel for nn_MultiHeadAttention (B=2, S=2048, D=1024, H=16).

Sharding: 8 cores = 2 batches x 4 head-groups. Core c handles batch c//4 and
heads [4*(c%4), 4*(c%4)+4). Each core computes its 4 heads' attention plus the
row-slice of the output projection; the host sums the 4 partial outputs per
batch and adds the output bias.

Per-core structure (all matmuls bf16, fp32 PSUM):
  - qT/kT in [head_dim, seq] layout (host provides x^T), packed 2 heads per
    128-partition tile. wq is host-prescaled by 16*log2(e) so the score PSUM
    is 128*log2(e)*scores/8 directly.
  - scoresT[kv, q] = kT.T @ qT per (head, kv-tile-pair, q-chunk); exp2 on
    ScalarE reading PSUM (scale=ln2/128, bias=-4*ln2 folded into the
    activation; the 2^-4 shift cancels in normalization and keeps bf16
    exponents comfortable).
  - attn[q, d] in q-major layout: attnT'[q, d+1] = ex_slice.T @ [v | 1]
    accumulated over the 16 kv tiles in PSUM (kv innermost so only one
    2KB accumulation region is live per (head, q-tile)); col 64 collects
    the softmax denominators.
  - 1/sums via exact DVE reciprocal; normalize+drain to bf16 on GPSIMD with
    a per-partition scalar multiply.
  - attn -> attnT via DMA XBAR transpose (head pairs stacked to 128 rows);
    out[q, :] = attnT.T @ woPair accumulated over the 2 head pairs (K=128).
"""

import sys

for _p in ("/opt/trn_rl_repo",):
    if _p not in sys.path:
        sys.path.insert(0, _p)

import numpy as np
import ml_dtypes

BF16 = ml_dtypes.bfloat16

S = 2048          # sequence length
D = 1024          # embed dim
HC = 4            # heads per core
HD = 64           # head dim
DC = HC * HD      # per-core projection width (256)
ST = S // 128     # kv tiles (16)
DT = D // 128     # D tiles (8)
QC = S // 512     # q chunks of 512 (4)
NCORES = 8

LOG2E = 1.4426950408889634
PRESCALE = 16.0 * LOG2E   # folded into wq on the host
CSHIFT = 4.0              # ex = exp(s/8) * 2^-CSHIFT

_PROGRAM = None


def _build_program():
    import concourse.mybir as mybir
    import concourse.tile as tile
    from concourse import bacc

    dt = mybir.dt
    AF = mybir.ActivationFunctionType
    ALU = mybir.AluOpType

    nc = bacc.Bacc()

    xqT = nc.declare_dram_parameter("xqT", [D, S], dt.bfloat16, isOutput=False)
    xkT = nc.declare_dram_parameter("xkT", [D, S], dt.bfloat16, isOutput=False)
    xvT = nc.declare_dram_parameter("xvT", [D, S], dt.bfloat16, isOutput=False)
    wq = nc.declare_dram_parameter("wq", [D, DC], dt.bfloat16, isOutput=False)
    wk = nc.declare_dram_parameter("wk", [D, DC], dt.bfloat16, isOutput=False)
    wv = nc.declare_dram_parameter("wv", [D, DC], dt.bfloat16, isOutput=False)
    # wo pairs: [pair, 128 = (2 heads x 64 hd), D]
    woP = nc.declare_dram_parameter("woP", [128, 2, D], dt.bfloat16, isOutput=False)
    bq = nc.declare_dram_parameter("bq", [128, 2], dt.float32, isOutput=False)
    bk = nc.declare_dram_parameter("bk", [128, 2], dt.float32, isOutput=False)
    bv = nc.declare_dram_parameter("bv", [128, DC], dt.float32, isOutput=False)
    out = nc.declare_dram_parameter("out", [S, D], dt.bfloat16, isOutput=True)

    out_t = out.rearrange("(t p) d -> t p d", p=128)

    EXP_SCALE = float(np.log(2.0) / 128.0)
    EXP_BIAS = float(-CSHIFT * np.log(2.0))

    with tile.TileContext(nc) as tc:
        with (
            tc.tile_pool(name="const", bufs=1) as cp,
            tc.tile_pool(name="xt", bufs=18) as xp,
            tc.tile_pool(name="xqt", bufs=16) as xqp,
            tc.tile_pool(name="expp", bufs=40) as ep,
            tc.tile_pool(name="atp", bufs=12) as atp,
            tc.tile_pool(name="attp", bufs=16) as atpT,
            tc.tile_pool(name="rcp", bufs=8) as rcp,
            tc.tile_pool(name="outp", bufs=4) as op_,
            tc.tile_pool(name="pa", bufs=2, space="PSUM") as pa,
            tc.tile_pool(name="pv", bufs=2, space="PSUM") as pv,
            tc.tile_pool(name="pb", bufs=2, space="PSUM") as pb,
        ):
            # ---- constants ----
            wq_sb = cp.tile([128, DT, DC], dt.bfloat16, tag="wq_sb")
            wk_sb = cp.tile([128, DT, DC], dt.bfloat16, tag="wk_sb")
            wv_sb = cp.tile([128, DT, DC], dt.bfloat16, tag="wv_sb")
            wo_sb = cp.tile([128, 2, D], dt.bfloat16, tag="wo_sb")
            bq_sb = cp.tile([128, 2], dt.float32, tag="bq_sb")
            bk_sb = cp.tile([128, 2], dt.float32, tag="bk_sb")
            bv_sb = cp.tile([128, HC, HD], dt.float32, tag="bv_sb")
            ebias_sb = cp.tile([128, 1], dt.float32, tag="ebias_sb")
            nc.gpsimd.memset(ebias_sb[:], EXP_BIAS)
            warm_sb = cp.tile([1, 1], dt.float32, tag="warm_sb")
            nc.gpsimd.memset(warm_sb[:], 0.0)
            # tiny warm-up exp so the activation-table load happens at t~0
            # instead of stalling the first real exp tile
            nc.scalar.activation(warm_sb[:], warm_sb[:], AF.Exp)
            nc.sync.dma_start(wk_sb[:], wk.rearrange("(t p) m -> p t m", p=128))
            nc.sync.dma_start(bk_sb[:], bk[:])

            qT_sb = [cp.tile([128, 2, 512], dt.bfloat16, tag=f"qT_sb{i}", name=f"qT_sb{i}") for i in range(QC)]
            kT_sb = [cp.tile([128, 2, 512], dt.bfloat16, tag=f"kT_sb{i}", name=f"kT_sb{i}") for i in range(QC)]
            # v' per kv tile: [128 kv, head, 64 v + ones col]
            v_sb = [cp.tile([128, HC, 65], dt.bfloat16, tag=f"v_sb{i}", name=f"v_sb{i}") for i in range(ST)]
            for st in range(ST):
                nc.gpsimd.memset(v_sb[st][:, :, 64:65], 1.0)

            # ---- projection helpers (x staged as [128, 512] quarter tiles) ----
            def load_xquarter(xT, xts, q, pool=None):
                xr = xT.rearrange("(t p) s -> p t s", p=128)
                for Dti in range(DT):
                    xtile = (pool or xp).tile([128, 512], dt.bfloat16, tag="xt",
                                              name=f"xt_{Dti}_{q}")
                    nc.sync.dma_start(xtile[:], xr[:, Dti, q * 512:(q + 1) * 512])
                    xts[Dti][q] = xtile

            def load_xhalf2(xT, xts, qlo, pool=None):
                xr = xT.rearrange("(t p) s -> p t s", p=128)
                for Dti in range(DT):
                    xtile = (pool or xp).tile([128, 1024], dt.bfloat16, tag="xt",
                                              name=f"xh_{Dti}_{qlo}")
                    nc.sync.dma_start(xtile[:], xr[:, Dti, qlo * 512:(qlo + 2) * 512])
                    xts[Dti][qlo] = xtile[:, 0:512]
                    xts[Dti][qlo + 1] = xtile[:, 512:1024]

            def qk_proj(xts, w_sb, dst, b_sb, qc):
                for pt in range(2):
                    ps = pb.tile([128, 512], dt.float32, tag="pb", name=f"pp_{qc}_{pt}")
                    for Dti in range(DT):
                        nc.tensor.matmul(
                            ps[:],
                            w_sb[:, Dti, pt * 128:(pt + 1) * 128],
                            xts[Dti][qc][:],
                            start=(Dti == 0),
                            stop=(Dti == DT - 1),
                        )
                    nc.vector.tensor_scalar_add(
                        dst[qc][:, pt, :], ps[:], b_sb[:, pt:pt + 1],
                    )

            def v_proj(xts, st_range):
                for st in st_range:
                    q, off = st // 4, (st % 4) * 128
                    ps = pv.tile([128, DC], dt.float32, tag="pv", name=f"vp_{st}")
                    for Dti in range(DT):
                        nc.tensor.matmul(
                            ps[:],
                            xts[Dti][q][:, off:off + 128],
                            wv_sb[:, Dti, :],
                            start=(Dti == 0),
                            stop=(Dti == DT - 1),
                        )
                    nc.vector.tensor_tensor(
                        v_sb[st][:, :, 0:64],
                        ps.rearrange("p (h d) -> p h d", d=HD),
                        bv_sb[:],
                        ALU.add,
                    )

            # ---- attention ----
            ex_tiles = {}

            def scores_exp(qc, h, m):
                """Scores+exp for q-chunk qc, head h, kv-tile pair m (kv tiles 2m, 2m+1)."""
                pt, lo = h // 2, (h % 2) * 64
                scp = pa.tile([128, 2, 512], dt.float32, tag="pa", name=f"sc_{qc}_{h}_{m}")
                for j in range(2):
                    kt = m * 2 + j
                    nc.tensor.matmul(
                        scp[:, j, :],
                        kT_sb[kt // 4][lo:lo + 64, pt, (kt % 4) * 128:(kt % 4 + 1) * 128],
                        qT_sb[qc][lo:lo + 64, pt, :],
                        start=True,
                        stop=True,
                    )
                ex = ep.tile([128, 2, 512], dt.bfloat16, tag="ex", name=f"ex_{qc}_{h}_{m}")
                nc.scalar.activation(ex[:], scp[:], AF.Exp, scale=EXP_SCALE,
                                     bias=ebias_sb[:])
                ex_tiles[(qc, h, m)] = ex

            def attn_head(qc, h, at_pair, atT):
                """attnV + normalize for (qc, h); writes at_pair[qs][:, (h%2)*64:...].
                For the odd head of a pair, fires the pair's transpose per qs
                as soon as its half is written."""
                exs = [ex_tiles.pop((qc, h, m)) for m in range(8)]
                rc = rcp.tile([128, 4], dt.float32, tag="rc", name=f"rc_{qc}_{h}")
                pair = h // 2
                for qs in range(4):
                    pA = pv.tile([128, 512], dt.float32, tag="pv", name=f"att_{qc}_{h}_{qs}")
                    for kt in range(ST):
                        nc.tensor.matmul(
                            pA[:, 0:65],
                            exs[kt // 2][:, kt % 2, qs * 128:(qs + 1) * 128],
                            v_sb[kt][:, h, :],
                            start=(kt == 0),
                            stop=(kt == ST - 1),
                        )
                    nc.vector.reciprocal(rc[:, qs:qs + 1], pA[:, 64:65])
                    nc.vector.tensor_scalar_mul(
                        at_pair[qs][:, (h % 2) * 64:(h % 2) * 64 + 64],
                        pA[:, 0:64],
                        rc[:, qs:qs + 1],
                    )
                    if h % 2 == 1:
                        t = atpT.tile([128, 128], dt.bfloat16, tag="atT",
                                      name=f"atT_{qc}_{pair}_{qs}")
                        nc.sync.dma_start(t[:], at_pair[qs][:], transpose=True)
                        atT[(pair, qs)] = t
                        if qc == QC - 1 and h == 3 and qs >= 1:
                            out_proj(qc, qs - 1, atT)
                if qc == QC - 1 and h == 3:
                    out_proj(qc, 3, atT)

            def out_half(qc, qs, dc2, atT):
                st = qc * 4 + qs
                o_sb = op_.tile([128, 512], dt.bfloat16, tag="osb",
                                name=f"osb_{st}_{dc2}")
                po = pb.tile([128, 512], dt.float32, tag="pb", name=f"po_{st}_{dc2}")
                for pair in range(2):
                    nc.tensor.matmul(
                        po[:],
                        atT[(pair, qs)][:],
                        wo_sb[:, pair, dc2 * 512:(dc2 + 1) * 512],
                        start=(pair == 0),
                        stop=(pair == 1),
                    )
                nc.vector.tensor_copy(o_sb[:], po[:])
                nc.sync.dma_start(
                    out_t[st][:, dc2 * 512:(dc2 + 1) * 512], o_sb[:])

            def out_proj(qc, qs, atT):
                out_half(qc, qs, 0, atT)
                out_half(qc, qs, 1, atT)

            # ---- trace order ----
            # The one modeled HWDGE device serializes all DMAs in issue order:
            # weights, xk-q0/xq-q0 (first scores ~12us in), remaining kT
            # quarters, xq-q1 (feeds qproj1 -> the early qc1 scores batch that
            # bridges the vproj window), then xv, then xq q2/q3.
            xk_ts = [[None] * 4 for _ in range(DT)]
            xq_ts = [[None] * 4 for _ in range(DT)]
            xv_ts = [[None] * 4 for _ in range(DT)]
            nc.sync.dma_start(wq_sb[:], wq.rearrange("(t p) m -> p t m", p=128))
            nc.sync.dma_start(bq_sb[:], bq[:])
            load_xhalf2(xkT, xk_ts, 0)
            load_xquarter(xqT, xq_ts, 0, pool=xqp)
            load_xhalf2(xkT, xk_ts, 2)
            load_xquarter(xqT, xq_ts, 1, pool=xqp)
            nc.sync.dma_start(wv_sb[:], wv.rearrange("(t p) m -> p t m", p=128))
            nc.sync.dma_start(bv_sb[:], bv.rearrange("p (h d) -> p h d", d=HD))
            load_xhalf2(xvT, xv_ts, 0)
            load_xhalf2(xvT, xv_ts, 2)
            nc.sync.dma_start(wo_sb[:], woP[:])
            load_xhalf2(xqT, xq_ts, 2, pool=xqp)
            qk_proj(xk_ts, wk_sb, kT_sb, bk_sb, 0)
            qk_proj(xk_ts, wk_sb, kT_sb, bk_sb, 1)
            qk_proj(xq_ts, wq_sb, qT_sb, bq_sb, 0)
            # qc0 exp backlog: head 0 complete first (the ex ring frees in
            # exactly the order ah(0,*) consumes), m-paced behind kT arrivals
            for m in range(4):
                scores_exp(0, 0, m)
            qk_proj(xk_ts, wk_sb, kT_sb, bk_sb, 2)
            scores_exp(0, 0, 4)
            scores_exp(0, 0, 5)
            qk_proj(xk_ts, wk_sb, kT_sb, bk_sb, 3)
            scores_exp(0, 0, 6)
            scores_exp(0, 0, 7)
            for h in range(1, 4):
                for m in range(8):
                    scores_exp(0, h, m)

            # ---- software pipeline over (qc, h): scores for qc+1 stream
            # between attnV heads of qc so ACT never starves.
            def sc(qc, h, mlo=0, mhi=8):
                for m in range(mlo, mhi):
                    scores_exp(qc, h, m)

            atTs = {qc: {} for qc in range(QC)}
            ats = {}

            def ah(qc, h):
                pair = h // 2
                if (qc, pair) not in ats:
                    ats[(qc, pair)] = [
                        atp.tile([128, 128], dt.bfloat16, tag="at",
                                 name=f"at_{qc}_{pair}_{qs}")
                        for qs in range(4)
                    ]
                attn_head(qc, h, ats[(qc, pair)], atTs[qc])

            # Hand-rolled schedule. The tail interleaves the last two
            # scores batches at scp granularity ahead of the remaining attnV
            # heads so almost no PE work is queued behind the final exps.
            steps = [
                ("qp", 1), ("sc", 1, 0), ("vp",),
                ("ah", 0, 0), ("sc", 1, 1), ("qp", 2),
                ("ah", 0, 1), ("sc", 1, 2), ("qp", 3),
                ("ah", 0, 2), ("sc", 1, 3),
                ("ah", 0, 3), ("sc", 2, 0),
                ("ah", 1, 0), ("oh", 0, 0), ("sc", 2, 1), ("oh", 0, 1),
                ("ah", 1, 1), ("oh", 0, 2), ("sc", 2, 2), ("oh", 0, 3),
                ("ah", 1, 2), ("oh", 0, 4), ("sc", 2, 3), ("oh", 0, 5),
                ("ah", 1, 3), ("oh", 0, 6), ("sc", 3, 0), ("oh", 0, 7),
                ("ah", 2, 0), ("oh", 1, 0), ("sc", 3, 1), ("oh", 1, 1),
                ("ah", 2, 1), ("oh", 1, 2), ("sc", 3, 2, 0, 4), ("oh", 1, 3),
                ("ah", 2, 2), ("oh", 1, 4), ("sc", 3, 2, 4, 8),
                ("sc", 3, 3, 0, 2), ("oh", 1, 5),
                ("ah", 2, 3), ("oh", 1, 6), ("sc", 3, 3, 2, 5), ("oh", 1, 7),
                ("ah", 3, 0), ("oh", 2, 0), ("sc", 3, 3, 5, 8), ("oh", 2, 1),
                ("ah", 3, 1), ("oh", 2, 2), ("oh", 2, 3),
                ("ah", 3, 2), ("oh", 2, 4), ("oh", 2, 5),
                ("oh", 2, 6), ("oh", 2, 7),
                ("ah", 3, 3),
            ]
            for step in steps:
                op = step[0]
                if op == "sc":
                    sc(*step[1:])
                elif op == "ah":
                    ah(step[1], step[2])
                elif op == "oh":
                    out_half(step[1], step[2] // 2, step[2] % 2, atTs[step[1]])
                elif op == "qp":
                    qk_proj(xq_ts, wq_sb, qT_sb, bq_sb, step[1])
                elif op == "vp":
                    v_proj(xv_ts, range(0, ST))

    nc.finalize()
    return nc


def _get_program():
    global _PROGRAM
    if _PROGRAM is None:
        _PROGRAM = _build_program()
    return _PROGRAM


def _prep_core_inputs(x_q, x_k, x_v, wq, bq, wk, bk, wv, bv, wo):
    """Build the 8 per-core input dicts (host-side shard + cast)."""
    xT = {}
    for b in range(2):
        xT[b] = (
            np.ascontiguousarray(x_q[b].T).astype(BF16),
            np.ascontiguousarray(x_k[b].T).astype(BF16),
            np.ascontiguousarray(x_v[b].T).astype(BF16),
        )
    wq_s = (wq * PRESCALE).astype(BF16)
    bq_s = (bq * PRESCALE).astype(np.float32)
    in_maps = []
    for c in range(NCORES):
        b, g = c // 4, c % 4
        sl = slice(g * DC, (g + 1) * DC)
        # wo rows for this head group, head-pairs stacked on partitions:
        # woP[p, pair, :] with p = (h_in_pair * 64 + hd)
        wo_c = np.ascontiguousarray(
            wo[sl, :].reshape(2, 2 * HD, D).transpose(1, 0, 2)
        ).astype(BF16)
        in_maps.append({
            "xqT": xT[b][0],
            "xkT": xT[b][1],
            "xvT": xT[b][2],
            "wq": wq_s[:, sl],
            "wk": wk[:, sl].astype(BF16),
            "wv": wv[:, sl].astype(BF16),
            "woP": wo_c,
            "bq": np.ascontiguousarray(bq_s[sl].reshape(2, 128).T).astype(np.float32),
            "bk": np.ascontiguousarray(bk[sl].reshape(2, 128).T).astype(np.float32),
            "bv": np.broadcast_to(bv[sl], (128, DC)).astype(np.float32).copy(),
        })
    return in_maps


def kernel(x_q, x_k, x_v, wq, bq, wk, bk, wv, bv, wo, bo):
    from concourse.bass_utils import run_bass_kernel_spmd

    x_q = np.asarray(x_q, np.float32)
    x_k = np.asarray(x_k, np.float32)
    x_v = np.asarray(x_v, np.float32)
    wq = np.asarray(wq, np.float32)
    wk = np.asarray(wk, np.float32)
    wv = np.asarray(wv, np.float32)
    wo = np.asarray(wo, np.float32)
    bq = np.asarray(bq, np.float32)
    bk = np.asarray(bk, np.float32)
    bv = np.asarray(bv, np.float32)
    bo = np.asarray(bo, np.float32)

    nc = _get_program()
    in_maps = _prep_core_inputs(x_q, x_k, x_v, wq, bq, wk, bk, wv, bv, wo)
    res = run_bass_kernel_spmd(nc, in_maps, list(range(NCORES)))

    out = np.zeros((2, S, D), np.float32)
    for c in range(NCORES):
        out[c // 4] += res.results[c]["out"].astype(np.float32)
    out += bo
    return out
